# revision 14
# baseline (speedup 1.0000x reference)
"""Multi-head attention (B=2, S=2048, E=1024, H=16) on 8 TRN2 NeuronCores.

Sharding: tensor-parallel on heads - core c computes heads {2c, 2c+1} end to
end (QKV projection slice, attention, and the row-parallel slice of out_proj
over its 2 heads' 128 contraction dims), and returns a partial [4096, 1024]
fp16 output; the host sums the 8 partials and adds b_out.

Engine budget per repeat (the kernel is balanced across PE/ACT/DVE):
  PE:  QKV 98.3k cyc + scores 131.1k (2 quadrant-packed K=64 matmuls per
       k-chunk, concurrent on HW via tile_position) + PV 66.6k in the
       *swapped* orientation (es [k,q] stationary, [v|1] moving, N=65 - full
       128-wide output partitions, half the issue cycles of the [65,512]
       orientation) + out_proj 32.8k.
  ACT: exp of 13/16 of the score stream (Exp activation, scale=1/A16).
  DVE: fast-exp of 3/16 of the score stream via the Schraudolph int16 trick
       (q is pre-scaled by A16/8 at QKV eviction so bits = round(sc + B16),
       clamped at 0, written through an int16 bitcast of the bf16 es tile),
       QKV bias evictions, PV normalize (reciprocal + per-partition scalar
       multiply - the swapped PV puts softmax sums in the q-partition
       column), and out_proj psum eviction to fp16.
  DMA: v and attn transposes ride the DMA xbar (dma_start_transpose),
       replacing PE transposes + DVE copies.

Per-phase structure (8 phases = 2 batches x 4 q-blocks of 512): scores for
phase i+1 are emitted before PV of phase i so ACT/DVE always have exp work
queued; the next repeat's QKV projection runs inside the current repeat's
tail; vT is double-buffered.
"""
import sys

sys.path.insert(0, "/opt/trn_rl_repo")
import numpy as np
import ml_dtypes
import concourse.bass as bass
import concourse.mybir as mybir
import concourse.tile as tile
from concourse import bacc
from concourse.bass_utils import run_bass_kernel_spmd

P = 128
B = 2
S = 2048          # sequence length per batch
T = B * S         # 4096 global tokens
E = 1024
H = 16
D = 64            # head dim
NCORES = 8
EC = E // P       # 8 contraction chunks for QKV
NCH = 3           # feature chunks: q | k | v of the head pair
QB = 512          # q block (phase) size
NQB = S // QB     # 4 q blocks per batch
KC = S // P       # 16 k chunks per batch
TB = 512          # token block for streaming xT
NTB = T // TB     # 8
EKC = 2           # kc chunks grouped per es tile

LOG2E = 1.4426950408889634
A16 = 128.0 * LOG2E                      # bf16-exponent units per exp-arg
B16 = 127.0 * 128.0 - 0.04303 * 128.0    # Schraudolph bias (mean-centered)
QSCALE = A16 * 0.125                     # folded into q at QKV eviction
DVE_KCS = frozenset((4, 9, 14))          # k-chunks fast-exp'd on DVE

F32 = mybir.dt.float32
FP16 = mybir.dt.float16
BF16 = mybir.dt.bfloat16
I16 = mybir.dt.int16

_COMPILED = None


def build(repeat=1):
    nc = bacc.Bacc(None, target_bir_lowering=False)
    xT_d = nc.dram_tensor("xT", [P, EC, T], BF16, kind="ExternalInput")
    w_inT_d = nc.dram_tensor("w_inT", [P, EC, NCH * P], BF16, kind="ExternalInput")
    b_in_d = nc.dram_tensor("b_in", [P, NCH], F32, kind="ExternalInput")
    w_outT_d = nc.dram_tensor("w_outT", [P, E], BF16, kind="ExternalInput")
    out_d = nc.dram_tensor("out", [T // P, P, E], FP16, kind="ExternalOutput")

    add = mybir.AluOpType.add
    mult = mybir.AluOpType.mult
    amax = mybir.AluOpType.max

    with tile.TileContext(nc) as tc:
        with (
            tc.tile_pool(name="const", bufs=1) as const,
            tc.tile_pool(name="qk_p", bufs=2) as qk_p,
            tc.tile_pool(name="vt_p", bufs=2) as vt_p,
            tc.tile_pool(name="attn_p", bufs=4) as attn_p,
            tc.tile_pool(name="attnT_p", bufs=3) as attnT_p,
            tc.tile_pool(name="outp", bufs=3) as outp,
            tc.tile_pool(name="small", bufs=4) as small,
            tc.tile_pool(name="qkv_in", bufs=2) as qkv_in,
            tc.tile_pool(name="stg_p", bufs=4) as stg_p,
            tc.tile_pool(name="es_pool", bufs=17) as es_pool,
            tc.tile_pool(name="psum", bufs=1, space="PSUM") as psum,
        ):
            b_in_sb = const.tile([P, NCH], F32)
            nc.sync.dma_start(b_in_sb[:], b_in_d[:])
            w_outT_sb = const.tile([P, E], BF16)
            nc.sync.dma_start(w_outT_sb[:], w_outT_d[:])
            w_inT_sb = const.tile([P, EC, NCH * P], BF16)
            nc.sync.dma_start(w_inT_sb[:], w_inT_d[:])

            def emit_tb_proj(tbi, qk, js):
                # QKV projection matmuls + eviction for feature chunks js
                # of token block tbi.  Returns v_sb when js includes 2.
                xt, v_sb = qk[2][tbi], None
                if xt is None:
                    xt = qkv_in.tile([P, EC, TB], BF16, name="xt")
                    nc.sync.dma_start(xt[:],
                                      xT_d[:, :, tbi * TB:(tbi + 1) * TB])
                    qk[2][tbi] = xt
                for j in js:                # q, k, v
                    acc = psum.tile([P, TB], F32, name="work", bufs=2)
                    for ec in range(EC):
                        nc.tensor.matmul(
                            acc[:],
                            w_inT_sb[:, ec, j * P:(j + 1) * P],
                            xt[:, ec, :],
                            start=(ec == 0),
                            stop=(ec == EC - 1),
                        )
                    if j == 2:
                        v_sb = qkv_in.tile([P, TB], BF16, name="vsb")
                    nc.vector.tensor_scalar(
                        out=(qk[j][:, tbi * TB:(tbi + 1) * TB]
                             if j < 2 else v_sb[:]),
                        in0=acc[:],
                        scalar1=b_in_sb[:, j:j + 1],
                        scalar2=(QSCALE if j == 0 else None),
                        op0=add,
                        op1=(mult if j == 0 else mybir.AluOpType.bypass),
                    )
                return v_sb

            def emit_tb_vt(tbi, v_sb, vT):
                # the xbar transpose needs a base-partition-0 source and a
                # flat output; stage [128,128] then scatter on gpsimd (the
                # Pool engine is otherwise idle)
                for kci in range(TB // P):
                    kcg = tbi * (TB // P) + kci     # global k chunk 0..31
                    stg = stg_p.tile([P, 2, D], BF16, name="vstg")
                    nc.sync.dma_start_transpose(
                        stg[:], v_sb[:, kci * P:(kci + 1) * P]
                    )
                    nc.gpsimd.tensor_copy(
                        vT[:, kcg // KC, kcg % KC, :, 0:D], stg[:]
                    )

            def emit_scores(b, qb, kcs, es_tiles, qk):
                q0 = b * S + qb * QB
                k0 = b * S
                for kc in kcs:
                    sc = psum.tile([P, 2, QB], F32, name="sc", bufs=2)
                    for hi in range(2):
                        nc.tensor.matmul(
                            sc[:, hi, :],
                            qk[1][hi * D:(hi + 1) * D,
                                  k0 + kc * P:k0 + (kc + 1) * P],
                            qk[0][hi * D:(hi + 1) * D, q0:q0 + QB],
                            start=True, stop=True,
                            tile_position=(hi * D, 0),
                        )
                    if kc % EKC == 0:
                        es_tiles[kc // EKC] = es_pool.tile(
                            [P, EKC, 2, QB], BF16, name="es"
                        )
                    dst = es_tiles[kc // EKC][:, kc % EKC, :, :]
                    if kc in DVE_KCS:
                        nc.vector.tensor_scalar(
                            out=dst.bitcast(I16), in0=sc[:, :, :],
                            scalar1=B16, scalar2=0.0, op0=add, op1=amax,
                        )
                    else:
                        nc.scalar.activation(
                            dst, sc[:, :, :],
                            mybir.ActivationFunctionType.Exp,
                            scale=1.0 / A16,
                        )

            def emit_pv_qc(b, qc, es_tiles, attn_T, vT):
                # one psum tile (= one bank) per head: two accumulation
                # groups sharing a bank lose updates on HW
                inv = small.tile([P, 2, 1], F32, name="inv")
                attn_sb = attn_p.tile([P, 2, D], BF16, name="attn")
                for hi in range(2):
                    pv = psum.tile([P, D + 1], F32, name=f"pv{hi}", bufs=1)
                    for kc in range(KC):
                        nc.tensor.matmul(
                            pv[:],
                            es_tiles[kc // EKC][:, kc % EKC, hi,
                                                qc * P:(qc + 1) * P],
                            vT[:, b, kc, hi, :],
                            start=(kc == 0),
                            stop=(kc == KC - 1),
                        )
                    nc.vector.reciprocal(inv[:, hi, :], pv[:, D:D + 1])
                    nc.vector.tensor_scalar(
                        out=attn_sb[:, hi, :],
                        in0=pv[:, 0:D],
                        scalar1=inv[:, hi, :],
                        scalar2=None,
                        op0=mult,
                    )
                nc.sync.dma_start_transpose(
                    attn_T[:, qc * P:(qc + 1) * P], attn_sb[:]
                )

            def emit_outproj_tc(b, qb, tci, attn_T, out_sb):
                # all 4 token chunks share one osb tile; a single DMA ships
                # the phase's [4, 128, E] block after tc3
                for eb in range(E // 512):
                    op = psum.tile([P, 512], F32, name="work", bufs=2)
                    nc.tensor.matmul(
                        op[:],
                        attn_T[:, tci * P:(tci + 1) * P],
                        w_outT_sb[:, eb * 512:(eb + 1) * 512],
                        start=True, stop=True,
                    )
                    nc.vector.tensor_copy(
                        out_sb[:, tci, eb * 512:(eb + 1) * 512], op[:]
                    )
                if tci == 3:
                    tc_g0 = (b * S + qb * QB) // P
                    nc.sync.dma_start(
                        out_d[tc_g0:tc_g0 + 4].rearrange("t p e -> p t e"),
                        out_sb[:],
                    )

            def new_qk():
                # [0]/[1]: q (pre-scaled by QSCALE) and k, feature-major
                # [128 = hi*64+d, token]; [2]: per-tb xt staging tiles
                return [qk_p.tile([P, T], BF16, name=f"qk{j}")
                        for j in range(2)] + [[None] * NTB]

            def emit_full_tb(tbi, qk, vT):
                v_sb = emit_tb_proj(tbi, qk, range(3))
                emit_tb_vt(tbi, v_sb, vT)

            def emit_xt_dma(tbi, qk):
                if qk[2][tbi] is None:
                    xt = qkv_in.tile([P, EC, TB], BF16, name="xt")
                    nc.sync.dma_start(xt[:],
                                      xT_d[:, :, tbi * TB:(tbi + 1) * TB])
                    qk[2][tbi] = xt

            # ---- prologue: repeat 0's full QKV + phase 0's scores ----
            phases = [(b, qb) for b in range(B) for qb in range(NQB)]
            vT_cur = vt_p.tile([P, B, KC, 2, D + 1], BF16, name="vT")
            nc.vector.memset(vT_cur[:, :, :, :, D:D + 1], 1.0)
            qk_cur = new_qk()
            for tbi in range(NTB):
                emit_full_tb(tbi, qk_cur, vT_cur)
            es_next = {}
            emit_scores(0, 0, range(KC), es_next, qk_cur)

            # out_proj trails PV by one phase so it never waits on the
            # attn transpose DMA latency; prev_op = (b, qb, attn_T, out_sb)
            prev_op = None

            def emit_op(tci):
                if prev_op is not None:
                    emit_outproj_tc(prev_op[0], prev_op[1], tci,
                                    prev_op[2], prev_op[3])

            for r in range(repeat):
                last = r == repeat - 1
                if not last:
                    vT_nxt = vt_p.tile([P, B, KC, 2, D + 1], BF16, name="vT")
                    nc.vector.memset(vT_nxt[:, :, :, :, D:D + 1], 1.0)
                    qk_nxt = new_qk()
                for i, (b, qb) in enumerate(phases):
                    es_cur = es_next
                    es_next = {}
                    attn_T = attnT_p.tile([P, QB], BF16, name="attnT")
                    # scores target: next phase (wrapping into next repeat)
                    if i < 7:
                        nb, nqb = phases[i + 1]
                        s_qk, emit_s = qk_cur, True
                    else:
                        nb, nqb = phases[0]
                        s_qk, emit_s = (qk_nxt, True) if not last else (None, False)
                    # interleave scores / PV / out_proj(prev) / next-repeat
                    # QKV so the PE has filler while exp drains the sc psum
                    if not last:
                        emit_xt_dma(i, qk_nxt)

                    def sc_chunk(kcs):
                        if emit_s:
                            emit_scores(nb, nqb, kcs, es_next, s_qk)

                    sc_chunk(range(0, 1))
                    emit_op(0)
                    sc_chunk(range(1, 2))
                    emit_pv_qc(b, 0, es_cur, attn_T, vT_cur)
                    sc_chunk(range(2, 3))
                    emit_op(1)
                    sc_chunk(range(3, 4))
                    emit_pv_qc(b, 1, es_cur, attn_T, vT_cur)
                    sc_chunk(range(4, 5))
                    if not last:
                        emit_tb_proj(i, qk_nxt, (0,))
                    sc_chunk(range(5, 7))
                    emit_pv_qc(b, 2, es_cur, attn_T, vT_cur)
                    sc_chunk(range(7, 8))
                    emit_op(2)
                    sc_chunk(range(8, 9))
                    if not last:
                        emit_tb_proj(i, qk_nxt, (1,))
                    sc_chunk(range(9, 11))
                    emit_pv_qc(b, 3, es_cur, attn_T, vT_cur)
                    sc_chunk(range(11, 13))
                    if not last:
                        v_sb = emit_tb_proj(i, qk_nxt, (2,))
                    sc_chunk(range(13, 16))
                    if not last:
                        emit_tb_vt(i, v_sb, vT_nxt)
                    emit_op(3)
                    prev_op = (b, qb, attn_T,
                               outp.tile([P, 4, E], FP16, name="osb"))
                if not last:
                    vT_cur = vT_nxt
                    qk_cur = qk_nxt
            for tci in range(4):    # flush the final phase's out_proj
                emit_op(tci)

    nc.compile()
    return nc


def _prep_inputs(x, w_in, b_in, w_out):
    x = np.ascontiguousarray(np.asarray(x, dtype=np.float32))
    w_in = np.asarray(w_in, dtype=np.float32)
    b_in = np.asarray(b_in, dtype=np.float32)
    w_out = np.asarray(w_out, dtype=np.float32)

    xT = np.ascontiguousarray(
        x.reshape(T, E).T.reshape(EC, P, T).transpose(1, 0, 2)
    ).astype(ml_dtypes.bfloat16)                # [128, EC, T]

    in_maps = []
    for c in range(NCORES):
        h0 = 2 * c
        rows = []
        brows = []
        for j in range(3):                      # q, k, v
            for hi in range(2):
                r0 = j * E + (h0 + hi) * D
                rows.append(w_in[r0:r0 + D])
                brows.append(b_in[r0:r0 + D])
        rows = np.concatenate(rows)             # [384, 1024]
        w_inT_c = np.ascontiguousarray(
            rows.T.reshape(EC, P, NCH * P).transpose(1, 0, 2)
        ).astype(ml_dtypes.bfloat16)            # [128, EC, 384]
        b_c = np.ascontiguousarray(
            np.concatenate(brows).reshape(NCH, P).T
        )                                       # [128, 3]
        # w_outT [128 = hi*64+d, E]
        w_outT_c = np.empty((P, E), dtype=np.float32)
        for hi in range(2):
            h = h0 + hi
            w_outT_c[hi * D:(hi + 1) * D] = w_out[:, h * D:(h + 1) * D].T
        w_outT_c = w_outT_c.astype(ml_dtypes.bfloat16)
        in_maps.append({
            "xT": xT,
            "w_inT": w_inT_c,
            "b_in": b_c,
            "w_outT": w_outT_c,
        })
    return in_maps


def kernel(x, w_in, b_in, w_out, b_out, _trace=False):
    global _COMPILED
    if _COMPILED is None:
        _COMPILED = build()
    nc = _COMPILED

    in_maps = _prep_inputs(x, w_in, b_in, w_out)
    res = run_bass_kernel_spmd(
        nc, in_maps, core_ids=list(range(NCORES)), trace=_trace
    )
    partial = np.zeros((T // P, P, E), dtype=np.float32)
    for c in range(NCORES):
        partial += res.results[c]["out"]
    out = partial.reshape(T, E) + np.asarray(b_out, dtype=np.float32)
    out = out.reshape(B, S, E)
    if _trace:
        return out, res
    return out
